# revision 4
# baseline (speedup 1.0000x reference)
"""Trainium2 Bass kernel for nn_BaseModel_55705725829328 (gnn_message_passing) — v2.

Math (forward only):
  M[b,j,t]   = 1{ log_alpha[j,t] + noise[b,j,t] > 0 }          (hard gumbel-sigmoid)
  u[b,j,t]   = M[b,j,t] * adj[j,t] * x[b,j]                     (adj = 1 - eye)
  h0[b,t,:]  = leaky_relu(W0[t] @ u[b,:,t] + b0[t])
  h1[b,t,:]  = leaky_relu(W1[t] @ h0[b,t,:] + b1[t])
  out[b,t,:] = W2[t] @ h1[b,t,:] + b2[t]

v2 design (vs the 111.7us b-major baseline):
  * noise ships as int16 (value*5800, f32 thresholds exact): halves the DMA and
    makes every DVE operand 2-byte. Mask flips from quantization cost ~2e-3
    rel err (gate is 2e-2).
  * t-major pipeline: 13 groups of 8 t's, full 512-row batch per group.
  * mask: per-t tensor_scalar is_gt (4x_2p DVE mode, threshold is the
    per-partition scalar) -> one grouped 2x tensor_tensor multiply by x.
    Compares are split DVE/Pool to balance engine load.
  * L0 matmul is swapped: out[b,(t,i)] = u_t^T @ W0_t with K=101 (a ones row
    in u folds the bias in). Output free size is 16, collapsing PE time.
  * lk0 is transposed back (4 PE transposes/group) so L1 contracts (t,i) on
    partitions with a block-diagonal W1; L2 block weights produce [16, 512]
    per group. Biases ride ACT activations as per-partition [P,1] APs.
  * output leaves as [(t,p), b]; the host transposes.

Sharding: data-parallel over batch across 8 cores (512 rows each).
"""

import os
import sys

sys.path.insert(0, "/opt/trn_rl_repo")

import numpy as np
from contextlib import ExitStack

import concourse.bass as bass
import concourse.mybir as mybir
from concourse.bass_utils import run_bass_kernel_spmd

# ---------------- problem constants (hardcoded per spec) ----------------
BS, D, H, P = 4096, 100, 16, 2
NCORES = 8
BC = BS // NCORES            # 512 batch rows per core
TG = 8                       # t's per group (L1 block-diag needs TG*H <= 128)
NGRP = (D + TG - 1) // TG    # 13 groups; last has 4 t's
SCALE = 5800.0               # int16 noise quantization scale
ALPHA = 0.01                 # leaky_relu negative slope (jax default)
NPOOL = 3                    # even-group gpsimd compare count (odd groups use 4)

F32 = mybir.dt.float32
FP16 = mybir.dt.float16
I16 = mybir.dt.int16


def _tg(g):
    return TG if g < NGRP - 1 else D - TG * (NGRP - 1)


def _npool(g):
    """compares per group issued on gpsimd; rest on DVE. Pool's per-op rate is
    ~4.2x DVE's 4x_2p rate, so the split balances engine busy. The first two
    groups lean on DVE, which would otherwise idle while Pool ramps."""
    if g == 0:
        return 0
    if g == 1:
        return 2
    if _tg(g) < TG:
        return 2
    return 4 if g % 2 else NPOOL


def _pool_ts(g):
    """t-indices within group g handled by gpsimd (tail of the group)."""
    n = _tg(g)
    k = min(_npool(g), max(0, n - 1))
    return list(range(n - k, n))


def _dve_ts(g):
    n = _tg(g)
    return [t for t in range(n) if t not in set(_pool_ts(g))]


def _cpp_total(g):
    """cumulative pool-compare count through group g (inclusive)."""
    return sum(len(_pool_ts(h)) for h in range(g + 1))


# blob layout (fp16 columns)
def _blob_layout():
    entries = [
        ("w0r", 101, D * H),         # [j | ones, (t, i)], row 100 = b0
        ("w1blk", 128, NGRP * 128),  # block-diag W1 per group
        ("w2blk", 128, NGRP * H),    # W2 block per group -> 16 out cols
        ("id128", 128, 128),         # fp16 identity for transposes
        ("b1c", 128, 2 * NGRP),      # f32 pairs: b1 per (th, i') row, col g
        ("b2c", 16, 2 * NGRP),       # f32 pairs: b2 per (th, p) row, col g
    ]
    lay = {}
    c = 0
    for name, rows, cols, in entries:
        lay[name] = (c, rows, cols)
        c += cols
    return lay, c


# ---------------- host-side prep ----------------

def _prep_shared(x, log_alpha, W0, b0, W1, b1, W2, b2):
    f32 = np.float32
    x = np.asarray(x, f32)
    la = np.asarray(log_alpha, f32)
    W0, b0 = np.asarray(W0, f32), np.asarray(b0, f32)
    W1, b1 = np.asarray(W1, f32), np.asarray(b1, f32)
    W2, b2 = np.asarray(W2, f32), np.asarray(b2, f32)

    thr = (-la) * f32(SCALE)
    np.fill_diagonal(thr, f32(1e9))          # no-self-loop mask

    lay, wtot = _blob_layout()
    blob = np.zeros((128, wtot), np.float16)

    def put(name, a):
        c, rows, cols = lay[name]
        blob[:rows, c:c + cols] = a

    def put_f32(name, a):
        c, rows, cols = lay[name]
        blob[:rows, c:c + cols] = np.ascontiguousarray(a.astype(f32)).view(np.float16)

    # w0r: [j, (t, i)] with bias row
    w0r = np.zeros((101, D * H), f32)
    w0r[:D] = np.transpose(W0, (2, 0, 1)).reshape(D, D * H)
    w0r[100] = b0.reshape(D * H)
    put("w0r", w0r.astype(np.float16))

    w1blk = np.zeros((128, NGRP * 128), f32)
    w2blk = np.zeros((128, NGRP * H), f32)
    b1c = np.zeros((128, NGRP), f32)
    b2c = np.zeros((16, NGRP), f32)
    for g in range(NGRP):
        for th in range(_tg(g)):
            t = TG * g + th
            w1blk[th * H:(th + 1) * H, g * 128 + th * H:g * 128 + (th + 1) * H] = W1[t].T
            w2blk[th * H:(th + 1) * H, g * H + th * P:g * H + (th + 1) * P] = W2[t].T
            b1c[th * H:(th + 1) * H, g] = b1[t]
            b2c[th * P:(th + 1) * P, g] = b2[t]
    put("w1blk", w1blk.astype(np.float16))
    put("w2blk", w2blk.astype(np.float16))
    put("id128", np.eye(128, dtype=np.float16))
    put_f32("b1c", b1c)
    put_f32("b2c", b2c)

    ones = np.ones((1, TG * BC), np.float16)
    return blob, thr, ones, lay


# ---------------- device program ----------------

def build_nc():
    nc = bass.Bass(detect_race_conditions=False)
    lay, wtot = _blob_layout()
    gt = mybir.AluOpType.is_gt
    mul = mybir.AluOpType.mult
    lrelu = mybir.ActivationFunctionType.Lrelu
    cpy = mybir.ActivationFunctionType.Copy

    nz_h = nc.dram_tensor("nz", [D, D, BC], I16, kind="ExternalInput")
    thr_h = nc.dram_tensor("thr", [D, D], F32, kind="ExternalInput")
    xe_h = nc.dram_tensor("xe", [D, BC], FP16, kind="ExternalInput")
    blob_h = nc.dram_tensor("cblob", [128, wtot], FP16, kind="ExternalInput")
    ones_h = nc.dram_tensor("ones", [1, TG * BC], FP16, kind="ExternalInput")
    out_h = nc.dram_tensor("out", [2 * D, BC], F32, kind="ExternalOutput")

    with ExitStack() as ctx:
        def sb(name, shape, dtype):
            return ctx.enter_context(nc.sbuf_tensor(name, shape, dtype))

        def ps(name, shape, dtype=F32):
            return ctx.enter_context(nc.psum_tensor(name, shape, dtype))

        blob_t = sb("blob_t", [128, wtot], FP16)
        ths = sb("ths", [D, D], F32)
        x_t = sb("x_t", [D, BC], FP16)
        nzb = [sb(f"nzb{i}", [D, TG * BC], I16) for i in range(4)]
        cs = [sb(f"cs{i}", [D, TG * BC], FP16) for i in range(4)]
        ub = [sb(f"ub{i}", [101, TG * BC], FP16) for i in range(2)]
        lk0 = [sb(f"lk0_{i}", [128, BC], FP16) for i in range(2)]
        a0T = [sb(f"a0T_{i}", [128, BC], FP16) for i in range(2)]
        lk1 = [sb(f"lk1_{i}", [128, BC], FP16) for i in range(2)]
        sbo = [sb(f"sbo{i}", [16, BC], F32) for i in range(4)]

        psL0 = [ps(f"psL0_{i}", [128, BC]) for i in range(2)]
        psT = [ps(f"psT_{i}", [128, BC], FP16) for i in range(2)]
        psL1 = [ps(f"psL1_{i}", [128, BC]) for i in range(2)]
        psL2 = [ps(f"psL2_{i}", [16, BC]) for i in range(2)]

        s_w = ctx.enter_context(nc.semaphore("s_w"))
        s_nz = ctx.enter_context(nc.semaphore("s_nz"))
        s_cpd = ctx.enter_context(nc.semaphore("s_cpd"))
        s_cpp = ctx.enter_context(nc.semaphore("s_cpp"))
        s_u = ctx.enter_context(nc.semaphore("s_u"))
        s_pe = ctx.enter_context(nc.semaphore("s_pe"))
        s_act = ctx.enter_context(nc.semaphore("s_act"))
        s_dr = ctx.enter_context(nc.semaphore("s_dr"))
        s_pd = ctx.enter_context(nc.semaphore("s_pd"))
        s_out = ctx.enter_context(nc.semaphore("s_out"))

        def bview(name):
            c, rows, cols = lay[name]
            return blob_t[0:rows, c:c + cols]

        def bview_f32(name):
            c, rows, cols = lay[name]
            return blob_t[0:rows, c:c + cols].bitcast(F32)

        w0r_t = bview("w0r")
        w1_t = bview("w1blk")
        w2_t = bview("w2blk")
        id_t = bview("id128")
        b1c_t = bview_f32("b1c")
        b2c_t = bview_f32("b2c")

        block = ctx.enter_context(nc.Block())

        # Skewed software pipeline: PE iter `it` runs L0(it), T(it-1),
        # L1(it-2), L2(it-3); ACT trails PE within the iteration. PE never
        # waits on same-iteration ACT work, so groups stream at engine rate
        # instead of serializing on the 8-stage per-group chain.
        # The final group's halves are skewed a further iteration apart so the
        # A-half chain is never queued behind B-half work on PE.
        pe_v, act_v = {}, {}
        GL = NGRP - 1
        cnum = 0
        for it in range(NGRP + 4):
            if it < NGRP:
                for k in (("L0A", "L0B") if it == GL else ("L0",)):
                    cnum += 1; pe_v[(k, it)] = cnum
            if 0 <= it - 1 < NGRP:
                for k in (("TA", "TB") if it - 1 == GL else ("T",)):
                    cnum += 1; pe_v[(k, it - 1)] = cnum
            if 0 <= it - 2 < NGRP:
                for k in (("L1A",) if it - 2 == GL else ("L1",)):
                    cnum += 1; pe_v[(k, it - 2)] = cnum
            if 0 <= it - 3 < NGRP:
                for k in (("L2A", "L1B") if it - 3 == GL else ("L2",)):
                    cnum += 1; pe_v[(k, it - 3)] = cnum
            if it - 4 == GL:
                cnum += 1; pe_v[("L2B", GL)] = cnum
        # The last groups' ACT stages that DVE can express run on DVE (idle
        # after its last mul) to shorten the serial drain chain; s_dr signals.
        # The final group is processed in two b-halves (A: cols 0:256,
        # B: 256:512) so its stage chain pipelines at half-width.
        DRAIN = {("a0T", NGRP - 2): 1, ("lr0A", GL): 2, ("lr0B", GL): 3,
                 ("a0TA", GL): 4, ("lr1A", GL): 5, ("a0TB", GL): 6,
                 ("lr1B", GL): 7, ("sboA", GL): 8, ("sboB", GL): 9}
        # (gpsimd cannot access PSUM, so no drain stages can go to Pool)
        POOL_DRAIN = {}
        cnum = 0
        for it in range(NGRP + 3):
            if it < NGRP and it != GL and ("lr0", it) not in POOL_DRAIN:
                cnum += 1; act_v[("lr0", it)] = cnum
            if 0 <= it - 1 < NGRP and ("a0T", it - 1) not in DRAIN \
                    and it - 1 != GL:
                cnum += 1; act_v[("a0T", it - 1)] = cnum
            if 0 <= it - 2 < NGRP and it - 2 != GL:
                cnum += 1; act_v[("lr1", it - 2)] = cnum
            if 0 <= it - 3 < NGRP and ("sbo", it - 3) not in DRAIN \
                    and it - 3 != GL:
                cnum += 1; act_v[("sbo", it - 3)] = cnum

        def stage_wait(eng, stage):
            if stage in DRAIN:
                eng.wait_ge(s_dr, DRAIN[stage])
            elif stage in POOL_DRAIN:
                eng.wait_ge(s_pd, POOL_DRAIN[stage])
            else:
                eng.wait_ge(s_act, act_v[stage])

        @block.gpsimd
        def _(gpsimd):
            gpsimd.wait_ge(s_w, 16)                # thr
            for h in range(NGRP):
                gpsimd.wait_ge(s_nz, 32 if h == 0 else 16 * (h + 2))
                if h >= 2:
                    gpsimd.wait_ge(s_u, h - 1)     # mul(h-2) freed cs[h%2]
                for th in _pool_ts(h):
                    t = TG * h + th
                    nc.gpsimd.tensor_scalar(
                        out=cs[h % 4][:, th * BC:(th + 1) * BC],
                        in0=nzb[h % 4][:, th * BC:(th + 1) * BC],
                        scalar1=ths[:, t:t + 1], scalar2=None, op0=gt,
                    ).then_inc(s_cpp, 1)


        @block.vector
        def _(vector):
            vector.wait_ge(s_w, 16)                # thr
            for g in range(NGRP):
                gw = _tg(g) * BC
                vector.wait_ge(s_nz, 16 if g == 0 else 16 * (g + 2))
                for th in _dve_ts(g):
                    if g == 0 and th == 4:
                        vector.wait_ge(s_nz, 32)   # second half of group 0
                    t = TG * g + th
                    ins = nc.vector.tensor_scalar(
                        out=cs[g % 4][:, th * BC:(th + 1) * BC],
                        in0=nzb[g % 4][:, th * BC:(th + 1) * BC],
                        scalar1=ths[:, t:t + 1], scalar2=None, op0=gt,
                    )
                ins.then_inc(s_cpd, 1)
                vector.wait_ge(s_cpp, _cpp_total(g))
                if g == 0:
                    vector.wait_ge(s_w, 32)        # x loaded
                if g >= 2:
                    vector.wait_ge(s_pe, pe_v[("L0", g - 2)])  # L0(g-2) freed ub[g%2]
                xa = x_t[:]
                xb = bass.AP(xa.tensor, xa.offset, [xa.ap[0], [0, _tg(g)], xa.ap[-1]])
                nc.vector.tensor_tensor(
                    out=ub[g % 2][0:D, 0:gw], in0=cs[g % 4][0:D, 0:gw], in1=xb,
                    op=mul,
                ).then_inc(s_u, 1)

            # drain assist: the whole final-group chain runs on DVE in two
            # b-halves so it never queues behind older groups' ACT stages.
            g2, g1 = NGRP - 2, GL
            A, B = slice(0, 256), slice(256, BC)

            def lr0_half(cl):
                # stt cannot read two PSUM operands; copy to SBUF first
                nc.vector.tensor_copy(out=lk0[g1 % 2][:, cl],
                                      in_=psL0[g1 % 2][:, cl])
                nc.vector.scalar_tensor_tensor(
                    out=lk0[g1 % 2][:, cl], in0=lk0[g1 % 2][:, cl],
                    scalar=ALPHA, in1=lk0[g1 % 2][:, cl], op0=mul,
                    op1=mybir.AluOpType.max).then_inc(s_dr, 1)

            def lr1_half(cl):
                nc.vector.tensor_scalar(
                    out=lk1[g1 % 2][:, cl], in0=psL1[g1 % 2][:, cl],
                    scalar1=b1c_t[:, g1:g1 + 1], scalar2=None,
                    op0=mybir.AluOpType.add)
                nc.vector.scalar_tensor_tensor(
                    out=lk1[g1 % 2][:, cl], in0=lk1[g1 % 2][:, cl],
                    scalar=ALPHA, in1=lk1[g1 % 2][:, cl], op0=mul,
                    op1=mybir.AluOpType.max).then_inc(s_dr, 1)

            def sbo_half(cl):
                nc.vector.tensor_scalar(
                    out=sbo[g1 % 4][:, cl], in0=psL2[g1 % 2][:, cl],
                    scalar1=b2c_t[:, g1:g1 + 1], scalar2=None,
                    op0=mybir.AluOpType.add).then_inc(s_dr, 1)

            vector.wait_ge(s_pe, pe_v[("T", g2)])
            nc.vector.tensor_copy(out=a0T[g2 % 2][:], in_=psT[g2 % 2][:]
                                  ).then_inc(s_dr, 1)                  # 1
            vector.wait_ge(s_pe, pe_v[("L0A", g1)])
            lr0_half(A)                                                # 2
            vector.wait_ge(s_pe, pe_v[("L0B", g1)])
            lr0_half(B)                                                # 3
            vector.wait_ge(s_pe, pe_v[("TA", g1)])
            nc.vector.tensor_copy(out=a0T[g1 % 2][:, A], in_=psT[g1 % 2][:, A]
                                  ).then_inc(s_dr, 1)                  # 4
            vector.wait_ge(s_pe, pe_v[("L1A", g1)])
            lr1_half(A)                                                # 5
            vector.wait_ge(s_pe, pe_v[("TB", g1)])
            nc.vector.tensor_copy(out=a0T[g1 % 2][:, B], in_=psT[g1 % 2][:, B]
                                  ).then_inc(s_dr, 1)                  # 6
            vector.wait_ge(s_pe, pe_v[("L1B", g1)])
            lr1_half(B)                                                # 7
            vector.wait_ge(s_pe, pe_v[("L2A", g1)])
            sbo_half(A)                                                # 8
            vector.wait_ge(s_pe, pe_v[("L2B", g1)])
            sbo_half(B)                                                # 9

        @block.tensor
        def _(tensor):
            tensor.wait_ge(s_w, 80)                # w0r + ones rows
            for it in range(NGRP + 4):
                if it == 1:
                    tensor.wait_ge(s_w, 96)        # id128 (transposes) loaded
                if it < NGRP:
                    g = it
                    u = ub[g % 2]
                    tensor.wait_ge(s_u, g + 1)
                    if g >= 2:
                        tensor.wait_ge(s_act, act_v[("lr0", g - 2)])  # psL0 free
                    for half in range(2 if g == GL else 1):
                        cr = (range(2), range(2, 4))[half] if g == GL else range(4)
                        last = None
                        for c in cr:
                            for th in range(_tg(g)):
                                t = TG * g + th
                                last = nc.tensor.matmul(
                                    out=psL0[g % 2][:, c * 128 + th * H:c * 128 + (th + 1) * H],
                                    lhsT=u[0:101, th * BC + c * 128:th * BC + (c + 1) * 128],
                                    rhs=w0r_t[0:101, t * H:(t + 1) * H],
                                    start=True, stop=True, skip_group_check=True,
                                )
                        last.then_inc(s_pe, 1)
                if 0 <= it - 1 < NGRP:
                    g = it - 1
                    if g >= 2 and ("lr0", g) in POOL_DRAIN or g == GL:
                        # drain waits lose ACT-order implications: guard the
                        # psT parity buffer against a0T(g-2) explicitly
                        tensor.wait_ge(s_act, act_v[("a0T", g - 2)])
                    for half in range(2 if g == GL else 1):
                        stage_wait(tensor, (("lr0A", "lr0B")[half], g)
                                   if g == GL else ("lr0", g))
                        cr = (range(2), range(2, 4))[half] if g == GL else range(4)
                        for c in cr:
                            last = nc.tensor.transpose(
                                psT[g % 2][:, c * 128:(c + 1) * 128],
                                lk0[g % 2][:, c * 128:(c + 1) * 128],
                                id_t,
                            )
                        last.then_inc(s_pe, 1)
                def l1_mm(g, half):
                    cl = slice(half * 256, half * 256 + 256) if g == GL \
                        else slice(0, BC)
                    nc.tensor.matmul(
                        out=psL1[g % 2][:, cl],
                        lhsT=w1_t[:, g * 128:(g + 1) * 128],
                        rhs=a0T[g % 2][:, cl], start=True, stop=True,
                        skip_group_check=True,
                    ).then_inc(s_pe, 1)

                def l2_mm(g, half):
                    cl = slice(half * 256, half * 256 + 256) if g == GL \
                        else slice(0, BC)
                    nc.tensor.matmul(
                        out=psL2[g % 2][0:16, cl],
                        lhsT=w2_t[:, g * H:(g + 1) * H],
                        rhs=lk1[g % 2][:, cl], start=True, stop=True,
                        skip_group_check=True,
                    ).then_inc(s_pe, 1)

                if 0 <= it - 2 < NGRP:
                    g = it - 2
                    if g >= 2 and (("a0T", g) in DRAIN or g == GL):
                        tensor.wait_ge(s_act, act_v[("lr1", g - 2)])  # psL1 free
                    if g == GL:
                        stage_wait(tensor, ("a0TA", g))
                        l1_mm(g, 0)
                    else:
                        stage_wait(tensor, ("a0T", g))
                        l1_mm(g, 0)
                if 0 <= it - 3 < NGRP:
                    g = it - 3
                    if g == GL:
                        tensor.wait_ge(s_act, act_v[("sbo", g - 2)])  # psL2 free
                        stage_wait(tensor, ("lr1A", g))
                        l2_mm(g, 0)
                        stage_wait(tensor, ("a0TB", g))
                        l1_mm(g, 1)
                    else:
                        stage_wait(tensor, ("lr1", g))
                        l2_mm(g, 0)
                if it - 4 == GL:
                    stage_wait(tensor, ("lr1B", GL))
                    l2_mm(GL, 1)

        @block.scalar
        def _(scalar):
            for it in range(NGRP + 3):
                if it < NGRP and it != GL and ("lr0", it) not in POOL_DRAIN:
                    g = it
                    scalar.wait_ge(s_pe, pe_v[("L0", g)])
                    nc.scalar.activation(lk0[g % 2][:], psL0[g % 2][:], lrelu,
                                         alpha=ALPHA).then_inc(s_act, 1)
                if 0 <= it - 1 < NGRP and ("a0T", it - 1) not in DRAIN \
                        and it - 1 != GL:
                    g = it - 1
                    scalar.wait_ge(s_pe, pe_v[("T", g)])
                    nc.scalar.activation(a0T[g % 2][:], psT[g % 2][:], cpy
                                         ).then_inc(s_act, 1)
                if 0 <= it - 2 < NGRP and it - 2 != GL:
                    g = it - 2
                    scalar.wait_ge(s_pe, pe_v[("L1", g)])
                    nc.scalar.activation(lk1[g % 2][:], psL1[g % 2][:], lrelu,
                                         alpha=ALPHA, bias=b1c_t[:, g:g + 1]
                                         ).then_inc(s_act, 1)
                if 0 <= it - 3 < NGRP and ("sbo", it - 3) not in DRAIN \
                        and it - 3 != GL:
                    g = it - 3
                    scalar.wait_ge(s_pe, pe_v[("L2", g)])
                    if g >= 4:
                        scalar.wait_ge(s_out, 16 * (g - 3))  # out-dma(g-4) freed sbo
                    nc.scalar.activation(sbo[g % 4][:], psL2[g % 2][:], lrelu,
                                         alpha=1.0, bias=b2c_t[:, g:g + 1]
                                         ).then_inc(s_act, 1)
                    scalar.dma_start(out=out_h[16 * g:16 * g + 2 * _tg(g), :],
                                     in_=sbo[g % 4][0:2 * _tg(g), :]
                                     ).then_inc(s_out, 16)
            # drain groups' out-DMAs (sbo computed on DVE)
            ogl = 2 * _tg(GL)
            stage_wait(scalar, ("sboA", GL))
            scalar.dma_start(out=out_h[16 * GL:16 * GL + ogl, 0:256],
                             in_=sbo[GL % 4][0:ogl, 0:256]).then_inc(s_out, 16)
            stage_wait(scalar, ("sboB", GL))
            scalar.dma_start(out=out_h[16 * GL:16 * GL + ogl, 256:BC],
                             in_=sbo[GL % 4][0:ogl, 256:BC]).then_inc(s_out, 16)

        @block.sync
        def _(sync):
            # All input DMAs ride SP HWDGE (Pool engine freed for compares;
            # out-DMAs issue from ACT so SP's waits can't block them).
            # s_w: thr=16, xe=32, blob=48, ones=64/80
            def nz_dma(g):
                sync.dma_start(
                    out=nzb[g % 4][:, 0:_tg(g) * BC],
                    in_=nz_h[:, TG * g:TG * g + _tg(g), :],
                ).then_inc(s_nz, 16)

            # group 0 in two halves so compares start earlier; the const blob
            # splits so w0r (L0 weights) lands early and nz2/nz3 aren't
            # queued behind the bulk of the weights on the DMA engines.
            w0r_end = lay["w0r"][0] + lay["w0r"][2]
            sync.dma_start(out=nzb[0][:, 0:4 * BC],
                           in_=nz_h[:, 0:4, :]).then_inc(s_nz, 16)
            sync.dma_start(out=ths[:], in_=thr_h[:]).then_inc(s_w, 16)
            sync.dma_start(out=nzb[0][:, 4 * BC:TG * BC],
                           in_=nz_h[:, 4:TG, :]).then_inc(s_nz, 16)
            sync.dma_start(out=x_t[:], in_=xe_h[:]).then_inc(s_w, 16)
            nz_dma(1)
            nz_dma(2)
            sync.dma_start(out=blob_t[0:101, 0:w0r_end],
                           in_=blob_h[0:101, 0:w0r_end]).then_inc(s_w, 16)
            sync.dma_start(out=ub[0][100:101, :], in_=ones_h[:]).then_inc(s_w, 16)
            sync.dma_start(out=ub[1][100:101, :], in_=ones_h[:]).then_inc(s_w, 16)
            nz_dma(3)
            sync.dma_start(out=blob_t[:, w0r_end:],
                           in_=blob_h[:, w0r_end:]).then_inc(s_w, 16)
            # s_w: thr=16 xe=32 w0r=48 ones=64/80 blob-rest=96
            for g in range(4, NGRP):
                sync.wait_ge(s_cpd, g - 3)              # DVE cmps(g-4) done
                sync.wait_ge(s_cpp, _cpp_total(g - 4))  # Pool cmps(g-4)
                nz_dma(g)

    return nc


_NC_CACHE = None


def kernel(x, log_alpha, noise, W0, b0, W1, b1, W2, b2):
    global _NC_CACHE
    blob, thr, ones, lay = _prep_shared(x, log_alpha, W0, b0, W1, b1, W2, b2)

    noise = np.asarray(noise, np.float32)
    nzq = np.clip(np.rint(noise * np.float32(SCALE)), -32767, 32767).astype(np.int16)
    x = np.asarray(x, np.float32)

    in_maps = []
    for c in range(NCORES):
        in_maps.append({
            "nz": np.ascontiguousarray(nzq[c * BC:(c + 1) * BC].transpose(1, 2, 0)),
            "thr": thr,
            "xe": np.ascontiguousarray(x[c * BC:(c + 1) * BC].T.astype(np.float16)),
            "cblob": blob,
            "ones": ones,
        })

    if _NC_CACHE is None:
        _NC_CACHE = build_nc()
    nc = _NC_CACHE

    trace = os.environ.get("KERNEL_TRACE", "0") == "1"
    res = run_bass_kernel_spmd(nc, in_maps, core_ids=list(range(NCORES)), trace=trace)
    if trace and res.exec_time_ns is not None:
        print(f"HW exec time: {res.exec_time_ns} ns")

    # device rows: 16*g + 2*th + p  for t = 8g+th
    t_arr = np.arange(D)
    row = (16 * (t_arr // TG)[:, None] + 2 * (t_arr % TG)[:, None]
           + np.arange(P)[None, :])                       # [D, P]
    out = np.empty((BS, D, P), np.float32)
    for c in range(NCORES):
        dev = res.results[c]["out"]                       # [200, 512]
        out[c * BC:(c + 1) * BC] = dev[row].transpose(2, 0, 1)
    return out
